# revision 1
# baseline (speedup 1.0000x reference)
"""nn_CausalWanSelfAttention Trainium2 kernel (8-core SPMD, single launch).

Entry point: kernel(**inputs) -> np.ndarray [1, 6240, 1536] float32.

Strategy:
  - Phase A, token-sharded (780 tokens/core): q/k/v projections as float32r
    matmuls (near-fp32 accuracy at bf16 speed), rmsnorm via ones-matmul
    partition reduction, 3D-RoPE on pair-de-interleaved channels (host
    permutes W_q/W_k output channels so rotation pairs are partition-
    contiguous; the permutation cancels inside q.k dot products). With unit
    gains the per-token rmsnorm scale folds into the rope tables once per
    projection, skipping the per-tile normalization multiply.
  - One AllGather ships (k^T, v) in bf16 to every core; one small AllReduce
    combines the routing means phi_q/phi_k.
  - Top-2-of-4 chunk routing computed on device; per-head chunk indices are
    loaded into registers and the selected chunks' K/V blocks are gathered
    with dynamically-addressed DMA.
  - Phase B, query-sharded (own 780 queries x all 12 heads): logits^T
    matmuls (keys on partitions), fused exp+scale+pad-kill bias on the
    scalar engine, softmax denominator on the vector engine + ones-matmul
    reduction, PV accumulation in PSUM; the row-parallel output projection
    reads o^T straight from SBUF so it pipelines behind the per-head
    attention, writing this core's exact [780, 1536] output slice.
"""

from contextlib import ExitStack

import numpy as np

import concourse.bacc as bacc
import concourse.mybir as mybir
import concourse.tile as tile

F32R = mybir.dt.float32r
F32 = mybir.dt.float32
BF16 = mybir.dt.bfloat16

N_CORES = 8
S, D, NH, HD, C = 6240, 1536, 12, 128, 64
NT = D // 128          # 12 channel tiles
TOK = S // N_CORES     # 780 tokens per core
QB = TOK // 2          # 390 free-dim block
BLK = 896              # padded per-core token block (7*128)
NKT = BLK // 128       # 7 key tiles per block
NBLK = N_CORES         # 8 blocks
NCH = 4                # routing chunks
EPS = 1e-6
SM_SCALE = 1.0 / float(np.sqrt(HD))
KV_ELEMS = NT * 128 * BLK  # = BLK * D
XWDT = F32R  # dtype of x / weight operands (DMA-volume experiment knob)
SKIP_ROPE = False   # timing probe: replace rope with a copy
SKIP_EXP = False    # timing probe: replace exp (ACT) with DVE copy
SKIP_STMM = False   # timing probe: drop S^T and PV matmuls
MTAIL = TOK - 6 * 128  # 12


def build_kernel(n_cores=N_CORES, debug_outs=False, solo=False, phases="full", gather=True, unit_gains=True):
    nc = bacc.Bacc("TRN2", target_bir_lowering=False, debug=False,
                   num_devices=n_cores)

    xT = nc.dram_tensor("xT", [NT, 128, TOK], XWDT, kind="ExternalInput")
    wqT = nc.dram_tensor("wqT", [NT, 128, D], XWDT, kind="ExternalInput")
    wkT = nc.dram_tensor("wkT", [NT, 128, D], XWDT, kind="ExternalInput")
    wvT = nc.dram_tensor("wvT", [NT, 128, D], XWDT, kind="ExternalInput")
    woT = nc.dram_tensor("woT", [NT, 128, D], XWDT, kind="ExternalInput")
    gq = nc.dram_tensor("gq", [1, D], F32R, kind="ExternalInput")
    gk = nc.dram_tensor("gk", [1, D], F32R, kind="ExternalInput")
    fr = nc.dram_tensor("fr", [C, TOK], F32, kind="ExternalInput")
    fi = nc.dram_tensor("fi", [C, TOK], F32, kind="ExternalInput")
    chmask = nc.dram_tensor("chmask", [128, NCH], F32, kind="ExternalInput")
    tailbias = nc.dram_tensor("tailbias", [128, 1], F32, kind="ExternalInput")

    out = nc.dram_tensor("out", [TOK, D], F32, kind="ExternalOutput")
    dbg = {}
    if debug_outs:
        dbg["qT"] = nc.dram_tensor("dbg_qT", [NT, 128, TOK], F32, kind="ExternalOutput")
        dbg["kT"] = nc.dram_tensor("dbg_kT", [NT, 128, TOK], F32, kind="ExternalOutput")
        dbg["scores"] = nc.dram_tensor("dbg_scores", [1, NH * NCH], F32, kind="ExternalOutput")
        dbg["gates"] = nc.dram_tensor("dbg_gates", [1, NH * NCH], F32, kind="ExternalOutput")
        dbg["oT"] = nc.dram_tensor("dbg_oT", [128, NT, TOK], F32, kind="ExternalOutput")

    # collective buffers
    ag_in = nc.dram_tensor("ag_in", [2, KV_ELEMS], BF16)
    ag_out = nc.dram_tensor("ag_out", [NBLK, 2, KV_ELEMS], BF16, addr_space="Shared")
    phi_in = nc.dram_tensor("phi_in", [128, NT, 1 + NCH], F32)
    phi_out = nc.dram_tensor("phi_out", [128, NT, 1 + NCH], F32, addr_space="Shared")

    k_in_view = ag_in.ap()[0].rearrange("(t p n) -> t p n", p=128, n=BLK)
    v_in_view = ag_in.ap()[1].rearrange("(k d) -> k d", d=D)

    ones_col_t = nc.inline_tensor(np.ones((128, 1), np.float32), name="ones_col")
    ones_row_t = nc.inline_tensor(np.ones((1, 128), np.float32), name="ones_row")

    with tile.TileContext(nc) as tc, ExitStack() as top:
        consts = top.enter_context(tc.tile_pool(name="consts", bufs=1))
        ones_col = consts.tile([128, 1], F32R)
        nc.sync.dma_start(out=ones_col, in_=ones_col_t.ap().bitcast(F32R))
        ones_row = consts.tile([1, 128], F32R)
        nc.sync.dma_start(out=ones_row, in_=ones_row_t.ap().bitcast(F32R))
        gq_sb = consts.tile([1, D], F32R)
        nc.sync.dma_start(out=gq_sb, in_=gq[:, :])
        gk_sb = consts.tile([1, D], F32R)
        nc.sync.dma_start(out=gk_sb, in_=gk[:, :])
        fr_sb = consts.tile([C, TOK], F32)
        nc.sync.dma_start(out=fr_sb, in_=fr[:, :])
        fi_sb = consts.tile([C, TOK], F32)
        nc.sync.dma_start(out=fi_sb, in_=fi[:, :])
        cm_sb = consts.tile([128, NCH], F32)
        nc.sync.dma_start(out=cm_sb, in_=chmask[:, :])
        eps_sb = consts.tile([1, 1], F32)
        nc.vector.memset(eps_sb, EPS)
        tb_sb = consts.tile([128, 1], F32)
        nc.sync.dma_start(out=tb_sb, in_=tailbias[:, :])
        ones_bf = consts.tile([128, 1], BF16)
        nc.vector.memset(ones_bf, 1.0)

        # persistent across phases
        persist = top.enter_context(tc.tile_pool(name="persist", bufs=1))
        qbf = persist.tile([128, NT, TOK], BF16)
        phiq_sb = persist.tile([128, NT], F32)
        phik_sb = persist.tile([128, NT], F32)
        gbias = persist.tile([128, NH * NCH], F32)
        gbias_tail = persist.tile([128, NH * NCH], F32)

        # ---------------- Phase A ----------------
        with (
            tc.tile_pool(name="xin", bufs=1) as xin,
            tc.tile_pool(name="wts", bufs=12) as wts,
            tc.tile_pool(name="pa_mm", bufs=2, space="PSUM") as pa_mm,
            tc.tile_pool(name="pa_ss", bufs=2, space="PSUM") as pa_ss,
            tc.tile_pool(name="pa_g", bufs=1, space="PSUM") as pa_g,
            tc.tile_pool(name="raw", bufs=1) as rawp,
            tc.tile_pool(name="sqp", bufs=3) as sqp,
            tc.tile_pool(name="rope", bufs=3) as ropep,
            tc.tile_pool(name="ropet", bufs=1) as ropet,
            tc.tile_pool(name="zpads", bufs=1) as zpads,
            tc.tile_pool(name="outbf", bufs=3) as outbf,
            tc.tile_pool(name="small", bufs=2) as smallp,
            tc.tile_pool(name="frqp", bufs=2) as frqp,
        ):
            xT_sb = xin.tile([128, NT, TOK], XWDT)
            for k in range(NT):
                nc.sync.dma_start(out=xT_sb[:, k, :], in_=xT.ap()[k])

            # ---- v projection (natural layout [tok, ch]) ----
            zpad_v = zpads.tile([116, D], BF16, tag="zpadv")
            nc.vector.memset(zpad_v, 0.0)
            nc.sync.dma_start(out=v_in_view[TOK:BLK, :], in_=zpad_v)
            for nb in range(4):
                wv_nb = []
                for k in range(NT):
                    wt = wts.tile([128, 384], XWDT, tag="wt", name=f"wv{nb}_{k}")
                    nc.sync.dma_start(out=wt,
                                      in_=wvT.ap()[k, :, nb * 384:(nb + 1) * 384])
                    wv_nb.append(wt)
                for tb in range(7):
                    m = 128 if tb < 6 else MTAIL
                    pv = pa_mm.tile([128, 384], F32, tag="pmm", name=f"pv{nb}_{tb}")
                    for k in range(NT):
                        nc.tensor.matmul(
                            pv[:m, :],
                            lhsT=xT_sb[:, k, tb * 128: tb * 128 + m],
                            rhs=wv_nb[k],
                            start=(k == 0), stop=(k == NT - 1),
                        )
                    vbf = outbf.tile([128, 384], BF16, tag="vbf")
                    nc.vector.tensor_copy(out=vbf[:m, :], in_=pv[:m, :])
                    nc.sync.dma_start(
                        out=v_in_view[tb * 128: tb * 128 + m, nb * 384:(nb + 1) * 384],
                        in_=vbf[:m, :],
                    )

            # ---- q/k projections (transposed layout [ch, tok]) ----
            QSA = [(0, 512), (512, TOK - 512)]

            def qk_proj(wdram, g_sb, is_q):
                if not is_q:
                    zpad_k = zpads.tile([128, BLK - TOK], BF16, tag="zpadk")
                    nc.vector.memset(zpad_k, 0.0)
                    for t in range(NT):
                        nc.sync.dma_start(out=k_in_view[t, :, TOK:BLK], in_=zpad_k)
                raw = rawp.tile([128, NT, TOK], F32, tag="raw")
                psss = [pa_ss.tile([1, 512], F32, tag="pss", name=f"pss{i}")
                        for i in range(2)]
                for half in range(4):
                    w_half = []
                    for k in range(NT):
                        wt = wts.tile([128, 384], XWDT, tag="wt", name=f"w{half}_{k}")
                        nc.sync.dma_start(
                            out=wt, in_=wdram.ap()[k, :, half * 384:(half + 1) * 384])
                        w_half.append(wt)
                    for oi in range(3):
                        ot = half * 3 + oi
                        pk = pa_mm.tile([128, TOK], F32, tag="pmm")
                        for qi, (q0, qn) in enumerate(QSA):
                            for k in range(NT):
                                nc.tensor.matmul(
                                    pk[:, q0:q0 + qn],
                                    lhsT=w_half[k][:, oi * 128:(oi + 1) * 128],
                                    rhs=xT_sb[:, k, q0:q0 + qn],
                                    start=(k == 0), stop=(k == NT - 1),
                                )
                        nc.scalar.copy(out=raw[:, ot, :], in_=pk)
                        sq = sqp.tile([128, TOK], F32R, tag="sq")
                        nc.scalar.activation(out=sq, in_=raw[:, ot, :],
                                             func=mybir.ActivationFunctionType.Square)
                        for qi, (q0, qn) in enumerate(QSA):
                            nc.tensor.matmul(psss[qi][:, :qn], lhsT=ones_col,
                                             rhs=sq[:, q0:q0 + qn],
                                             start=(ot == 0), stop=(ot == NT - 1))
                rs = smallp.tile([1, TOK], F32R, tag="rs")
                for qi, (q0, qn) in enumerate(QSA):
                    rs1 = smallp.tile([1, 512], F32, tag="rs1")
                    nc.scalar.activation(out=rs1[:, :qn], in_=psss[qi][:, :qn],
                                         func=mybir.ActivationFunctionType.Sqrt,
                                         bias=eps_sb[0:1, 0:1], scale=1.0 / D)
                    with nc.allow_low_precision(reason="rms scale in f32r"):
                        nc.vector.reciprocal(out=rs[:, q0:q0 + qn], in_=rs1[:, :qn])
                if unit_gains:
                    # fold rs into the rope tables once per projection:
                    # rope(raw * rs) = raw x (fr*rs, fi*rs)
                    prs = pa_g.tile([128, TOK], F32, tag="pg", name="prs")
                    for qi, (q0, qn) in enumerate(QSA):
                        nc.tensor.matmul(prs[0:C, q0:q0 + qn],
                                         lhsT=ones_row[0:1, 0:C],
                                         rhs=rs[:, q0:q0 + qn], start=True, stop=True)
                    frq_t = frqp.tile([C, TOK], F32, tag="frq")
                    nc.vector.tensor_tensor(frq_t, fr_sb, prs[0:C, :],
                                            mybir.AluOpType.mult)
                    fiq_t = frqp.tile([C, TOK], F32, tag="fiq")
                    nc.vector.tensor_tensor(fiq_t, fi_sb, prs[0:C, :],
                                            mybir.AluOpType.mult)
                for ot in range(NT):
                    if unit_gains:
                        nrm = raw[:, ot, :]
                        frt, fit = frq_t, fiq_t
                    else:
                        pg = pa_g.tile([128, TOK], F32, tag="pg")
                        for qi, (q0, qn) in enumerate(QSA):
                            nc.tensor.matmul(pg[:, q0:q0 + qn],
                                             lhsT=g_sb[0:1, ot * 128:(ot + 1) * 128],
                                             rhs=rs[:, q0:q0 + qn], start=True, stop=True)
                        nrmt = sqp.tile([128, TOK], F32, tag="nrm")
                        nc.vector.tensor_tensor(nrmt, raw[:, ot, :], pg,
                                                mybir.AluOpType.mult)
                        nrm = nrmt
                        frt, fit = fr_sb, fi_sb
                    # rope: pairs de-interleaved -> a=rows 0:64, b=rows 64:128
                    a = nrm[0:C, :]
                    ro = ropep.tile([128, TOK], F32, tag="ro")
                    if SKIP_ROPE:
                        nc.vector.tensor_copy(out=ro, in_=nrm)
                    else:
                        bsh = ropet.tile([C, TOK], F32, tag="bsh")
                        nc.scalar.copy(out=bsh, in_=nrm[C:128, :])
                        t1 = ropet.tile([C, TOK], F32, tag="t1")
                        t2 = ropet.tile([C, TOK], F32, tag="t2")
                        rb = ropet.tile([C, TOK], F32, tag="rb")
                        nc.vector.tensor_tensor(t1, a, frt, mybir.AluOpType.mult)
                        nc.vector.tensor_tensor(t2, bsh, fit, mybir.AluOpType.mult)
                        nc.vector.tensor_tensor(ro[0:C, :], t1, t2, mybir.AluOpType.subtract)
                        nc.vector.tensor_tensor(t1, a, fit, mybir.AluOpType.mult)
                        nc.vector.tensor_tensor(t2, bsh, frt, mybir.AluOpType.mult)
                        nc.vector.tensor_tensor(rb, t1, t2, mybir.AluOpType.add)
                        nc.scalar.copy(out=ro[C:128, :], in_=rb)
                    phi_dst = phiq_sb if is_q else phik_sb
                    nc.vector.reduce_sum(out=phi_dst[:, ot: ot + 1], in_=ro,
                                         axis=mybir.AxisListType.X)
                    if is_q:
                        nc.scalar.copy(out=qbf[:, ot, :], in_=ro)
                        if debug_outs:
                            nc.sync.dma_start(out=dbg["qT"].ap()[ot], in_=ro)
                    else:
                        kbf = outbf.tile([128, TOK], BF16, tag="kbf")
                        nc.scalar.copy(out=kbf, in_=ro)
                        nc.sync.dma_start(out=k_in_view[ot, :, 0:TOK], in_=kbf)
                        if debug_outs:
                            nc.sync.dma_start(out=dbg["kT"].ap()[ot], in_=ro)

            qk_proj(wkT, gk_sb, is_q=False)

            # AllGather (kT, v) once k and v blocks are written
            if not solo:
                nc.gpsimd.collective_compute(
                    "AllGather", mybir.AluOpType.bypass,
                    replica_groups=[list(range(n_cores))],
                    ins=[ag_in.ap().opt()], outs=[ag_out.ap().opt()],
                )

            qk_proj(wqT, gq_sb, is_q=True)

            # ---- phi AllReduce ----
            nc.sync.dma_start(out=phi_in.ap()[:, :, 0:1],
                              in_=phiq_sb[:, :, None])
            phik_m = smallp.tile([128, NT, NCH], F32, tag="phikm")
            for ch in range(NCH):
                nc.vector.tensor_scalar_mul(phik_m[:, :, ch], phik_sb,
                                            cm_sb[:, ch: ch + 1])
            nc.sync.dma_start(out=phi_in.ap()[:, :, 1: 1 + NCH], in_=phik_m)
            if not solo:
                nc.gpsimd.collective_compute(
                    "AllReduce", mybir.AluOpType.add,
                    replica_groups=[list(range(n_cores))],
                    ins=[phi_in.ap().opt()], outs=[phi_out.ap().opt()],
                )

            # ---- routing scores + top-2 gates ----
            phis = smallp.tile([128, NT, 1 + NCH], F32, tag="phis")
            nc.sync.dma_start(out=phis, in_=(phi_in if solo else phi_out).ap())
            prod = smallp.tile([128, NT, NCH], F32R, tag="prodsc")
            for t in range(NT):
                nc.vector.tensor_scalar_mul(prod[:, t, :], phis[:, t, 1: 1 + NCH],
                                            phis[:, t, 0:1])
            psc = pa_ss.tile([1, NH * NCH], F32, tag="pss")
            nc.tensor.matmul(psc, lhsT=ones_col,
                             rhs=prod[:, :, :].rearrange("p t c -> p (t c)"),
                             start=True, stop=True)
            sc = smallp.tile([1, NH * NCH], F32, tag="sc")
            nc.vector.tensor_copy(out=sc, in_=psc)
            scv = sc[:, :].rearrange("p (h c) -> p h c", c=NCH)
            m1 = smallp.tile([1, NH], F32, tag="m1")
            nc.vector.reduce_max(out=m1, in_=scv, axis=mybir.AxisListType.X)
            is1 = smallp.tile([1, NH * NCH], F32, tag="is1")
            nc.vector.tensor_tensor(
                is1[:, :].rearrange("p (h c) -> p h c", c=NCH),
                scv, m1[:, :, None].to_broadcast((1, NH, NCH)),
                mybir.AluOpType.is_ge)
            nc.vector.tensor_scalar_mul(is1, is1, 1e30)
            masked = smallp.tile([1, NH * NCH], F32, tag="masked")
            nc.vector.tensor_tensor(masked, sc, is1, mybir.AluOpType.subtract)
            m2 = smallp.tile([1, NH], F32, tag="m2")
            nc.vector.reduce_max(out=m2,
                                 in_=masked[:, :].rearrange("p (h c) -> p h c", c=NCH),
                                 axis=mybir.AxisListType.X)
            gates = smallp.tile([1, NH * NCH], F32, tag="gates")
            nc.vector.tensor_tensor(
                gates[:, :].rearrange("p (h c) -> p h c", c=NCH),
                scv, m2[:, :, None].to_broadcast((1, NH, NCH)),
                mybir.AluOpType.is_ge)
            gb = smallp.tile([1, NH * NCH], F32R, tag="gb")
            with nc.allow_low_precision(reason="gate bias 0/-1e30"):
                nc.vector.tensor_scalar(gb, gates, 1e30, -1e30,
                                        mybir.AluOpType.mult, mybir.AluOpType.add)
            pgb = pa_g.tile([128, NH * NCH], F32, tag="pg")
            nc.tensor.matmul(pgb, lhsT=ones_row, rhs=gb, start=True, stop=True)
            nc.vector.tensor_copy(out=gbias, in_=pgb)
            nc.vector.tensor_scalar(gbias_tail, pgb, tb_sb[:, 0:1], None,
                                    mybir.AluOpType.add)
            if gather:
                # chunk indices: i1 = argmax, i2 = arg-2nd-max  (per head)
                iota4 = smallp.tile([1, NCH], F32, tag="iota4")
                nc.gpsimd.iota(iota4.bitcast(mybir.dt.int32), pattern=[[1, NCH]],
                               base=0, channel_multiplier=0)
                nc.vector.tensor_copy(out=iota4, in_=iota4.bitcast(mybir.dt.int32))
                is2 = smallp.tile([1, NH * NCH], F32, tag="is2")
                nc.vector.tensor_tensor(
                    is2[:, :].rearrange("p (h c) -> p h c", c=NCH),
                    masked[:, :].rearrange("p (h c) -> p h c", c=NCH),
                    m2[:, :, None].to_broadcast((1, NH, NCH)),
                    mybir.AluOpType.is_ge)
                nc.vector.tensor_scalar_mul(is1, is1, 1e-30)  # undo 1e30 scale -> 0/1
                idxf = smallp.tile([1, NH, 2], F32, tag="idxf")
                w1 = smallp.tile([1, NH * NCH], F32, tag="w1")
                nc.vector.tensor_tensor(
                    w1[:, :].rearrange("p (h c) -> p h c", c=NCH),
                    is1[:, :].rearrange("p (h c) -> p h c", c=NCH),
                    iota4[:, None, :].to_broadcast((1, NH, NCH)),
                    mybir.AluOpType.mult)
                nc.vector.reduce_sum(out=idxf[:, :, 0], in_=w1[:, :].rearrange(
                    "p (h c) -> p h c", c=NCH), axis=mybir.AxisListType.X)
                nc.vector.tensor_tensor(
                    w1[:, :].rearrange("p (h c) -> p h c", c=NCH),
                    is2[:, :].rearrange("p (h c) -> p h c", c=NCH),
                    iota4[:, None, :].to_broadcast((1, NH, NCH)),
                    mybir.AluOpType.mult)
                nc.vector.reduce_sum(out=idxf[:, :, 1], in_=w1[:, :].rearrange(
                    "p (h c) -> p h c", c=NCH), axis=mybir.AxisListType.X)
                idx_i32 = persist.tile([1, NH * 2], mybir.dt.int32)
                nc.vector.tensor_copy(out=idx_i32,
                                      in_=idxf[:, :, :].rearrange("p h s -> p (h s)"))
            else:
                idx_i32 = None
            if debug_outs:
                nc.sync.dma_start(out=dbg["scores"].ap(), in_=sc)
                nc.sync.dma_start(out=dbg["gates"].ap(), in_=gates)

        # ---------------- Phase B: attention ----------------
        otp = top.enter_context(tc.tile_pool(name="otp", bufs=1))
        oT_sb = otp.tile([128, NT, TOK], XWDT)
        if phases == "a":
            return _finish(nc)
        QS = [(0, 512), (512, TOK - 512)]  # bank-aligned query splits
        n_batt = 2 * 2 if gather else NBLK  # blocks attended per head
        wop = top.enter_context(tc.tile_pool(name="wo", bufs=12))
        wo_tiles = []
        for k in range(NT):
            wt = wop.tile([128, D], XWDT, tag="wo", name=f"wo{k}")
            nc.sync.dma_start(out=wt, in_=woT.ap()[k])
            wo_tiles.append(wt)
        with (
            tc.tile_pool(name="kv", bufs=4) as kvp,
            tc.tile_pool(name="ebf", bufs=5) as ep,
            tc.tile_pool(name="dacc", bufs=2) as dp,
            tc.tile_pool(name="bsm", bufs=4) as bsm,
            tc.tile_pool(name="pb_s", bufs=2, space="PSUM") as pb_s,
            tc.tile_pool(name="pb_d", bufs=1, space="PSUM") as pb_d,
            tc.tile_pool(name="pb_o", bufs=2, space="PSUM") as pb_o,
        ):
            n_mm = n_batt * NKT
            for h in range(NH):
                dens = []
                pos = []
                den = dp.tile([128, TOK], F32, tag="den")
                nc.vector.memset(den, 0.0)
                for qb in range(2):
                    pos.append(pb_o.tile([128, 512], F32, tag="po", name=f"po{qb}"))
                if gather:
                    blk_regs = []
                    for sel in range(2):
                        iv = nc.values_load(
                            idx_i32[0:1, h * 2 + sel: h * 2 + sel + 1],
                            min_val=0, max_val=NCH - 1,
                            skip_runtime_bounds_check=True)
                        blk_regs.append(iv)
                for bi in range(n_batt):
                    if gather:
                        from concourse.bass import ds as _ds
                        blk = blk_regs[bi // 2] * 2 + (bi % 2)
                        kv_b = (ag_in.ap() if solo
                                else ag_out.ap()[_ds(blk, 1)][0])
                        gcol = None
                    else:
                        b = bi
                        kv_b = ag_in.ap() if solo else ag_out.ap()[b]
                        gcol = h * NCH + b // 2
                    kT_b = kvp.tile([128, BLK], BF16, tag="kb")
                    nc.sync.dma_start(
                        out=kT_b,
                        in_=kv_b[0].rearrange("(t p n) -> t p n", p=128, n=BLK)[h])
                    V_b = kvp.tile([128, NKT, 128], BF16, tag="vb")
                    nc.sync.dma_start(
                        out=V_b,
                        in_=kv_b[1].rearrange("(n p d) -> p n d", p=128, d=D)
                        [:, :, h * 128:(h + 1) * 128])
                    for kt in range(NKT):
                        i_mm = bi * NKT + kt
                        ps = pb_s.tile([128, TOK], F32, tag="ps")
                        for qb, (q0, qn) in enumerate(QS):
                            nc.tensor.matmul(ps[:, q0:q0 + qn],
                                             lhsT=kT_b[:, kt * 128:(kt + 1) * 128],
                                             rhs=qbf[:, h, q0:q0 + qn],
                                             start=True, stop=True)
                        ebf = ep.tile([128, TOK], BF16, tag="e")
                        if SKIP_EXP:
                            nc.vector.tensor_copy(out=ebf, in_=ps)
                        else:
                            if gather:
                                bias_ap = tb_sb[:, 0:1] if kt == NKT - 1 else 0.0
                            else:
                                bias_ap = (gbias_tail if kt == NKT - 1 else gbias)[:, gcol: gcol + 1]
                            nc.scalar.activation(out=ebf, in_=ps,
                                                 func=mybir.ActivationFunctionType.Exp,
                                                 bias=bias_ap, scale=SM_SCALE)
                        nc.vector.tensor_tensor(den, den, ebf, mybir.AluOpType.add)
                        for qb, (q0, qn) in enumerate(QS):
                            nc.tensor.matmul(pos[qb][:, :qn], lhsT=V_b[:, kt, :],
                                             rhs=ebf[:, q0:q0 + qn],
                                             start=(i_mm == 0), stop=(i_mm == n_mm - 1))
                prb = pb_s.tile([128, TOK], F32, tag="ps", name=f"prb{h}")
                dr = dp.tile([128, TOK], F32R, tag="dr")
                nc.vector.tensor_copy(out=dr, in_=den)
                pdp = pb_d.tile([1, TOK], F32, tag="pd")
                for qb, (q0, qn) in enumerate(QS):
                    nc.tensor.matmul(pdp[:, q0:q0 + qn], lhsT=ones_col,
                                     rhs=dr[:, q0:q0 + qn], start=True, stop=True)
                rec = bsm.tile([1, TOK], F32R, tag="rec")
                with nc.allow_low_precision(reason="softmax denom"):
                    nc.vector.reciprocal(out=rec, in_=pdp)
                for qb, (q0, qn) in enumerate(QS):
                    nc.tensor.matmul(prb[:, q0:q0 + qn], lhsT=ones_row,
                                     rhs=rec[:, q0:q0 + qn], start=True, stop=True)
                rb_sb = bsm.tile([128, TOK], F32, tag="rbsb")
                nc.scalar.copy(out=rb_sb, in_=prb)
                for qb, (q0, qn) in enumerate(QS):
                    with nc.allow_low_precision(reason="oT in f32r"):
                        nc.vector.tensor_tensor(oT_sb[:, h, q0:q0 + qn],
                                                pos[qb][:, :qn],
                                                rb_sb[:, q0:q0 + qn],
                                                mybir.AluOpType.mult)
                if debug_outs:
                    nc.sync.dma_start(out=dbg["oT"].ap()[:, h, :],
                                      in_=oT_sb[:, h, :].bitcast(F32))

        # ---------------- out projection ----------------
        if phases == "ab":
            return _finish(nc)
        with (
            tc.tile_pool(name="osb", bufs=3) as osb,
            tc.tile_pool(name="po_mm", bufs=2, space="PSUM") as po_mm,
        ):
            for tb in range(7):
                m = 128 if tb < 6 else MTAIL
                for nb in range(3):
                    pO = po_mm.tile([128, 512], F32, tag="pO")
                    for k in range(NT):
                        nc.tensor.matmul(pO[:m, :],
                                         lhsT=oT_sb[:, k, tb * 128: tb * 128 + m],
                                         rhs=wo_tiles[k][:, nb * 512:(nb + 1) * 512],
                                         start=(k == 0), stop=(k == NT - 1))
                    ob = osb.tile([128, 512], F32, tag="ob")
                    nc.scalar.copy(out=ob[:m, :], in_=pO[:m, :])
                    nc.sync.dma_start(
                        out=out.ap()[tb * 128: tb * 128 + m, nb * 512:(nb + 1) * 512],
                        in_=ob[:m, :])

    return _finish(nc)


def _finish(nc):
    nc.compile()
    return nc


# ---------------- host-side prep ----------------

def _perm():
    p = np.arange(D).reshape(NH, C, 2)
    return np.concatenate([p[:, :, 0], p[:, :, 1]], axis=1).reshape(-1)


def make_fcis(freqs, grid_sizes):
    f, h, w = [int(v) for v in np.asarray(grid_sizes)[0]]
    c1 = C - 2 * (C // 3)
    c2 = C // 3
    fq = np.asarray(freqs, np.float32)
    ff = np.broadcast_to(fq[:f, None, None, :c1], (f, h, w, c1, 2))
    fh = np.broadcast_to(fq[None, :h, None, c1:c1 + c2], (f, h, w, c2, 2))
    fw = np.broadcast_to(fq[None, None, :w, c1 + c2:c1 + 2 * c2], (f, h, w, c2, 2))
    return np.concatenate([ff, fh, fw], axis=3).reshape(f * h * w, C, 2)


def host_prep(inputs):
    """inputs: the full reference input dict -> per-core in_maps."""
    x = np.asarray(inputs["x"], np.float32)
    freqs = np.asarray(inputs["freqs"], np.float32)
    grid_sizes = np.asarray(inputs["grid_sizes"])
    assert x.shape == (1, S, D)
    assert int(np.asarray(inputs["chunk_size"])) == S // NCH
    assert int(np.asarray(inputs["top_k"])) == 2

    perm = _perm()
    wq = np.asarray(inputs["wq"], np.float32)[perm]
    wk = np.asarray(inputs["wk"], np.float32)[perm]
    wv = np.asarray(inputs["wv"], np.float32)
    wo = np.asarray(inputs["wo"], np.float32)
    gqv = np.asarray(inputs["gq"], np.float32)[perm]
    gkv = np.asarray(inputs["gk"], np.float32)[perm]
    for b in ("bq", "bk", "bv", "bo"):
        assert not np.any(np.asarray(inputs[b])), f"nonzero bias {b} unsupported"

    xT = np.ascontiguousarray(x[0].T).reshape(NT, 128, S)
    wqT = np.ascontiguousarray(wq.T).reshape(NT, 128, D)
    wkT = np.ascontiguousarray(wk.T).reshape(NT, 128, D)
    wvT = np.ascontiguousarray(wv.T).reshape(NT, 128, D)
    woT = np.ascontiguousarray(wo.T).reshape(NT, 128, D)

    fcis = make_fcis(freqs, grid_sizes)  # [S, C, 2]
    frT = fcis[:, :, 0].T  # [C, S]
    fiT = fcis[:, :, 1].T


    tail_bias = np.zeros((128, 1), np.float32)
    tail_bias[MTAIL:] = -1e30
    in_maps = []
    for c in range(N_CORES):
        sl = slice(c * TOK, (c + 1) * TOK)
        cm = np.zeros((128, NCH), np.float32)
        cm[:, (c * TOK) // (S // NCH)] = 1.0
        in_maps.append({
            "xT": np.ascontiguousarray(xT[:, :, sl]),
            "wqT": wqT, "wkT": wkT, "wvT": wvT, "woT": woT,
            "gq": gqv[None, :], "gk": gkv[None, :],
            "fr": np.ascontiguousarray(frT[:, sl]),
            "fi": np.ascontiguousarray(fiT[:, sl]),
            "chmask": cm,
            "tailbias": tail_bias,
        })
    return in_maps


def assemble_out(results):
    return np.concatenate([r["out"] for r in results], axis=0)[None]


# ---------------- harness entry point ----------------

_CACHE = {}


def kernel(**inputs):
    import numpy as _np
    ug = bool(_np.all(_np.asarray(inputs["gq"]) == 1.0)
              and _np.all(_np.asarray(inputs["gk"]) == 1.0))
    key = ("nc", ug)
    if key not in _CACHE:
        _CACHE[key] = build_kernel(unit_gains=ug)
    nc = _CACHE[key]
    in_maps = host_prep(inputs)
    from concourse import bass_utils
    res = bass_utils.run_bass_kernel_spmd(
        nc, in_maps, core_ids=list(range(N_CORES)), trace=False)
    return assemble_out(res.results).astype(_np.float32)



# revision 20
# speedup vs baseline: 1.6890x; 1.6890x over previous
"""nn_CausalWanSelfAttention Trainium2 kernel (8-core SPMD, single launch).

Entry point: kernel(**inputs) -> np.ndarray [1, 6240, 1536] float32.

Strategy (token-sharded projections + replicated-head attention over own
queries):
  - Phase A (780 tokens/core): q/k/v projections as bf16 matmuls (x and all
    weights shipped bf16; PSUM accumulates f32). rmsnorm via DVE squares +
    ones-matmul partition reduction; the per-token rsqrt scale is folded into
    stacked 3D-RoPE tables ([fr;fi] and [fi;fr] over the pair-de-interleaved
    channel layout), so rope is 4 bf16 DVE ops per 128-channel tile writing
    the roped q/k directly.
  - One AllGather ships (k^T, v) in bf16 (780-token blocks, no padding) to
    every core; one small AllReduce combines the routing means phi_q/phi_k.
  - Top-2-of-4 chunk routing on device; per-head chunk indices drive
    dynamically-addressed DMA gathers that pack the two selected 1560-key
    chunks contiguously into 25 key tiles (3200 cols, 80 pad).
  - Phase B, per head over own 780 queries: logits^T matmuls (keys on
    partitions), exp on the scalar engine with a partition bias killing the
    pad keys of the last tile, denominator accumulated in bf16 on the vector
    engine (2x mode), PV accumulated in PSUM; normalization multiplies the
    PSUM PV output by a ones-matmul broadcast reciprocal straight into the
    bf16 o^T buffer feeding the row-parallel output projection.
"""

from contextlib import ExitStack

import numpy as np

import concourse.bacc as bacc
import concourse.mybir as mybir
import concourse.tile as tile
from concourse.bass import ds as _ds

F32R = mybir.dt.float32r
F32 = mybir.dt.float32
BF16 = mybir.dt.bfloat16
I32 = mybir.dt.int32

N_CORES = 8
S, D, NH, HD, C = 6240, 1536, 12, 128, 64
NT = D // 128          # 12 channel tiles
TOK = S // N_CORES     # 780 tokens per core
NCH = 4                # routing chunks
CHUNK = S // NCH       # 1560 keys per chunk
SEL = 2 * CHUNK        # 3120 selected keys per head
NKT = (SEL + 127) // 128   # 25 key tiles after packing
KPAD = NKT * 128 - SEL     # 80 pad keys; last tile has 48 real rows
LASTK = 128 - KPAD         # 48
EPS = 1e-6
SM_SCALE = 1.0 / float(np.sqrt(HD))
KV_ELEMS = TOK * D
MTAIL = TOK - 6 * 128  # 12 (token-tile tail for v/out projections)
QSA = [(0, 512), (512, TOK - 512)]


def build_kernel(n_cores=N_CORES, solo=False, phases="full", unit_gains=True):
    nc = bacc.Bacc("TRN2", target_bir_lowering=False, debug=False,
                   num_devices=n_cores)

    xT = nc.dram_tensor("xT", [NT, 128, TOK], BF16, kind="ExternalInput")
    wqT = nc.dram_tensor("wqT", [NT, 128, D], BF16, kind="ExternalInput")
    wkT = nc.dram_tensor("wkT", [NT, 128, D], BF16, kind="ExternalInput")
    wvT = nc.dram_tensor("wvT", [NT, 128, D], BF16, kind="ExternalInput")
    woT = nc.dram_tensor("woT", [NT, 128, D], BF16, kind="ExternalInput")
    gq = nc.dram_tensor("gq", [128, NT], F32, kind="ExternalInput")
    gk = nc.dram_tensor("gk", [128, NT], F32, kind="ExternalInput")
    f1 = nc.dram_tensor("f1", [128, TOK], F32, kind="ExternalInput")
    f2 = nc.dram_tensor("f2", [128, TOK], F32, kind="ExternalInput")
    chmask = nc.dram_tensor("chmask", [128, NCH], F32, kind="ExternalInput")
    padbias = nc.dram_tensor("padbias", [128, 1], F32, kind="ExternalInput")

    out = nc.dram_tensor("out", [TOK, D], F32, kind="ExternalOutput")

    # collective buffers
    ag_in = nc.dram_tensor("ag_in", [2, KV_ELEMS], BF16)
    ag_out = nc.dram_tensor("ag_out", [N_CORES, 2, KV_ELEMS], BF16,
                            addr_space="Shared")
    phi_in = nc.dram_tensor("phi_in", [128, NT, 1 + NCH], F32)
    phi_out = nc.dram_tensor("phi_out", [128, NT, 1 + NCH], F32,
                             addr_space="Shared")

    k_in_view = ag_in.ap()[0].rearrange("(t p n) -> t p n", p=128, n=TOK)
    v_in_view = ag_in.ap()[1].rearrange("(k d) -> k d", d=D)

    ones_col_t = nc.inline_tensor(np.ones((128, 1), np.float32), name="ones_col")
    ones_row_t = nc.inline_tensor(np.ones((1, 128), np.float32), name="ones_row")
    half_np = np.zeros((128, 2), np.float32)
    half_np[:64, 0] = 1.0
    half_np[64:, 1] = 1.0
    ones_half_t = nc.inline_tensor(half_np, name="ones_half")

    with tile.TileContext(nc) as tc, ExitStack() as top:
        consts = top.enter_context(tc.tile_pool(name="consts", bufs=1))
        ones_col = consts.tile([128, 1], F32R)
        nc.sync.dma_start(out=ones_col, in_=ones_col_t.ap().bitcast(F32R))
        ones_row = consts.tile([1, 128], F32R)
        nc.sync.dma_start(out=ones_row, in_=ones_row_t.ap().bitcast(F32R))
        ones_half = consts.tile([128, 2], F32R)
        nc.sync.dma_start(out=ones_half, in_=ones_half_t.ap().bitcast(F32R))
        f1_sb = consts.tile([128, TOK], F32)
        nc.sync.dma_start(out=f1_sb, in_=f1[:, :])
        f2_sb = consts.tile([128, TOK], F32)
        nc.sync.dma_start(out=f2_sb, in_=f2[:, :])
        cm_sb = consts.tile([128, NCH], F32)
        nc.sync.dma_start(out=cm_sb, in_=chmask[:, :])
        pb_sb = consts.tile([128, 1], F32)
        nc.sync.dma_start(out=pb_sb, in_=padbias[:, :])
        gq_sb = consts.tile([128, NT], F32)
        nc.sync.dma_start(out=gq_sb, in_=gq[:, :])
        gk_sb = consts.tile([128, NT], F32)
        nc.sync.dma_start(out=gk_sb, in_=gk[:, :])
        eps_sb = consts.tile([1, 1], F32)
        nc.vector.memset(eps_sb, EPS)
        ones_col_bf = consts.tile([128, 1], BF16)
        nc.vector.memset(ones_col_bf, 1.0)

        # persistent across phases
        persist = top.enter_context(tc.tile_pool(name="persist", bufs=1))
        qbf = persist.tile([128, NT, TOK], BF16)
        phiq_sb = persist.tile([128, NT], F32)
        phik_sb = persist.tile([128, NT], F32)
        idx_i32 = persist.tile([1, NH * 2], I32)

        # ---------------- Phase A ----------------
        with (
            tc.tile_pool(name="xin", bufs=1) as xin,
            tc.tile_pool(name="wp", bufs=2) as wp,
            tc.tile_pool(name="pmm", bufs=2, space="PSUM") as pmm,
            tc.tile_pool(name="ppv", bufs=2, space="PSUM") as ppv,
            tc.tile_pool(name="pss", bufs=1, space="PSUM") as pss,
            tc.tile_pool(name="raw", bufs=2) as rawp,
            tc.tile_pool(name="sqp", bufs=3) as sqp,
            tc.tile_pool(name="tab", bufs=2) as tabp,
            tc.tile_pool(name="rop", bufs=2) as ropep,
            tc.tile_pool(name="kbf", bufs=3) as kbfp,
            tc.tile_pool(name="vbf", bufs=3) as vbfp,
            tc.tile_pool(name="small", bufs=2) as smallp,
        ):
            xT_sb = xin.tile([128, NT, TOK], BF16)
            for k in range(NT):
                nc.sync.dma_start(out=xT_sb[:, k, :], in_=xT.ap()[k])

            def load_w(wdram, name):
                w_sb = wp.tile([128, NT, D], BF16, tag="w", name=name)
                for k in range(NT):
                    nc.sync.dma_start(out=w_sb[:, k, :], in_=wdram.ap()[k])
                return w_sb

            wk_sb = load_w(wkT, "wk")
            wv_sb = load_w(wvT, "wv")

            def qk_proj(w_sb, g_col, is_q):
                raw = rawp.tile([128, NT, TOK], BF16, tag="raw")
                psss = pss.tile([1, TOK], F32, tag="pss")
                for ot in range(NT):
                    pk = pmm.tile([128, TOK], F32, tag="pk")
                    for q0, qn in QSA:
                        for k in range(NT):
                            nc.tensor.matmul(
                                pk[:, q0:q0 + qn],
                                lhsT=w_sb[:, k, ot * 128:(ot + 1) * 128],
                                rhs=xT_sb[:, k, q0:q0 + qn],
                                start=(k == 0), stop=(k == NT - 1),
                            )
                    nc.scalar.copy(out=raw[:, ot, :], in_=pk)
                    sq = sqp.tile([128, TOK], BF16, tag="sq")
                    with nc.allow_low_precision(reason="squares in bf16"):
                        nc.vector.tensor_tensor(sq, raw[:, ot, :], raw[:, ot, :],
                                                mybir.AluOpType.mult)
                    for q0, qn in QSA:
                        nc.tensor.matmul(psss[:, q0:q0 + qn], lhsT=ones_col_bf,
                                         rhs=sq[:, q0:q0 + qn],
                                         start=(ot == 0), stop=(ot == NT - 1))
                rs1 = smallp.tile([1, TOK], F32, tag="rs1")
                nc.scalar.activation(out=rs1, in_=psss,
                                     func=mybir.ActivationFunctionType.Sqrt,
                                     bias=eps_sb[0:1, 0:1], scale=1.0 / D)
                rs = smallp.tile([1, TOK], F32R, tag="rs")
                with nc.allow_low_precision(reason="rms scale in f32r"):
                    nc.vector.reciprocal(out=rs, in_=rs1)
                prs = pmm.tile([128, TOK], F32, tag="pk", name="prs")
                for q0, qn in QSA:
                    nc.tensor.matmul(prs[:, q0:q0 + qn],
                                     lhsT=ones_row[0:1, :],
                                     rhs=rs[:, q0:q0 + qn], start=True, stop=True)
                f1q = tabp.tile([128, TOK], BF16, tag="t1")
                f2q = tabp.tile([128, TOK], BF16, tag="t2")
                with nc.allow_low_precision(reason="rope tables in bf16"):
                    nc.vector.tensor_tensor(f1q, f1_sb, prs, mybir.AluOpType.mult)
                    nc.vector.tensor_tensor(f2q, f2_sb, prs, mybir.AluOpType.mult)
                # tile 2j holds the cos-halves (a) of heads 2j/2j+1, tile
                # 2j+1 the sin-halves (b); rope mixes the two tiles with the
                # duplicated tables f1q=[fr;fr]*rs, f2q=[fi;fi]*rs.
                for j in range(NT // 2):
                    ta, tb = 2 * j, 2 * j + 1
                    ra = raw[:, ta, :]
                    rb = raw[:, tb, :]
                    if not unit_gains:
                        tmpa = ropep.tile([128, TOK], BF16, tag="ga")
                        tmpb = ropep.tile([128, TOK], BF16, tag="gb")
                        with nc.allow_low_precision(reason="gain in bf16"):
                            nc.vector.tensor_scalar_mul(tmpa, ra,
                                                        g_col[:, ta:ta + 1])
                            nc.vector.tensor_scalar_mul(tmpb, rb,
                                                        g_col[:, tb:tb + 1])
                        ra, rb = tmpa, tmpb
                    m1_ = ropep.tile([128, TOK], BF16, tag="m1")
                    m2_ = ropep.tile([128, TOK], BF16, tag="m2")
                    if is_q:
                        dsta = qbf[:, ta, :]
                        dstb = qbf[:, tb, :]
                    else:
                        kba = kbfp.tile([128, TOK], BF16, tag="kba")
                        kbb = kbfp.tile([128, TOK], BF16, tag="kbb")
                        dsta, dstb = kba, kbb
                    with nc.allow_low_precision(reason="rope in bf16"):
                        nc.vector.tensor_tensor(m1_, ra, f1q, mybir.AluOpType.mult)
                        nc.vector.tensor_tensor(m2_, rb, f2q, mybir.AluOpType.mult)
                        nc.vector.tensor_tensor(dsta, m1_, m2_,
                                                mybir.AluOpType.subtract)
                        nc.vector.tensor_tensor(m1_, ra, f2q, mybir.AluOpType.mult)
                        nc.vector.tensor_tensor(m2_, rb, f1q, mybir.AluOpType.mult)
                        nc.vector.tensor_tensor(dstb, m1_, m2_,
                                                mybir.AluOpType.add)
                    phi_dst = phiq_sb if is_q else phik_sb
                    nc.vector.reduce_sum(out=phi_dst[:, ta:ta + 1], in_=dsta,
                                         axis=mybir.AxisListType.X)
                    nc.vector.reduce_sum(out=phi_dst[:, tb:tb + 1], in_=dstb,
                                         axis=mybir.AxisListType.X)
                    if not is_q:
                        nc.sync.dma_start(out=k_in_view[ta, :, :], in_=dsta)
                        nc.sync.dma_start(out=k_in_view[tb, :, :], in_=dstb)

            qk_proj(wk_sb, gk_sb, is_q=False)

            # ---- v projection (natural layout [tok, ch]) ----
            for nb in range(3):
                for tb in range(7):
                    m = 128 if tb < 6 else MTAIL
                    pv = ppv.tile([128, 512], F32, tag="pv")
                    for k in range(NT):
                        nc.tensor.matmul(
                            pv[:m, :],
                            lhsT=xT_sb[:, k, tb * 128: tb * 128 + m],
                            rhs=wv_sb[:, k, nb * 512:(nb + 1) * 512],
                            start=(k == 0), stop=(k == NT - 1),
                        )
                    vbf = vbfp.tile([128, 512], BF16, tag="vbf")
                    nc.scalar.copy(out=vbf[:m, :], in_=pv[:m, :])
                    nc.sync.dma_start(
                        out=v_in_view[tb * 128: tb * 128 + m,
                                      nb * 512:(nb + 1) * 512],
                        in_=vbf[:m, :],
                    )

            # AllGather (kT, v) once k and v blocks are written
            if not solo:
                nc.gpsimd.collective_compute(
                    "AllGather", mybir.AluOpType.bypass,
                    replica_groups=[list(range(n_cores))],
                    ins=[ag_in.ap().opt()], outs=[ag_out.ap().opt()],
                )

            wq_sb = load_w(wqT, "wq")
            qk_proj(wq_sb, gq_sb, is_q=True)

            # ---- phi AllReduce ----
            nc.sync.dma_start(out=phi_in.ap()[:, :, 0:1],
                              in_=phiq_sb[:, :, None])
            phik_m = smallp.tile([128, NT, NCH], F32, tag="phikm")
            for ch in range(NCH):
                nc.vector.tensor_scalar_mul(phik_m[:, :, ch], phik_sb,
                                            cm_sb[:, ch: ch + 1])
            nc.sync.dma_start(out=phi_in.ap()[:, :, 1: 1 + NCH], in_=phik_m)
            if not solo:
                nc.gpsimd.collective_compute(
                    "AllReduce", mybir.AluOpType.add,
                    replica_groups=[list(range(n_cores))],
                    ins=[phi_in.ap().opt()], outs=[phi_out.ap().opt()],
                )

            # ---- routing scores + top-2 chunk indices per head ----
            phis = smallp.tile([128, NT, 1 + NCH], F32, tag="phis")
            nc.sync.dma_start(out=phis, in_=(phi_in if solo else phi_out).ap())
            prod = smallp.tile([128, NT, NCH], F32R, tag="prodsc")
            for t in range(NT):
                nc.vector.tensor_scalar_mul(prod[:, t, :], phis[:, t, 1: 1 + NCH],
                                            phis[:, t, 0:1])
            # row 0: partitions 0:64 = per-tile sums for even heads; row 1:
            # partitions 64:128 = odd heads. score(h,c) sums the head's a-tile
            # (2j) and b-tile (2j+1) entries of its row.
            psc_ev = pss.tile([1, NT * NCH], F32, tag="pss", name="psc_ev")
            nc.tensor.matmul(psc_ev, lhsT=ones_half[:, 0:1],
                             rhs=prod[:, :, :].rearrange("p t c -> p (t c)"),
                             start=True, stop=True)
            psc_od = pss.tile([1, NT * NCH], F32, tag="pss", name="psc_od")
            nc.tensor.matmul(psc_od, lhsT=ones_half[:, 1:2],
                             rhs=prod[:, :, :].rearrange("p t c -> p (t c)"),
                             start=True, stop=True)
            sc_ev = smallp.tile([1, NT * NCH], F32, tag="scev")
            nc.scalar.copy(out=sc_ev, in_=psc_ev)
            sc_od = smallp.tile([1, NT * NCH], F32, tag="scod")
            nc.scalar.copy(out=sc_od, in_=psc_od)
            sc = smallp.tile([1, NH * NCH], F32, tag="sc")
            scp = sc[:, :].rearrange("p (j two c) -> p j two c", two=2, c=NCH)
            nc.vector.reduce_sum(
                out=scp[:, :, 0, :],
                in_=sc_ev[:, :].rearrange("p (j t c) -> p j c t", t=2, c=NCH),
                axis=mybir.AxisListType.X)
            nc.vector.reduce_sum(
                out=scp[:, :, 1, :],
                in_=sc_od[:, :].rearrange("p (j t c) -> p j c t", t=2, c=NCH),
                axis=mybir.AxisListType.X)
            scv = sc[:, :].rearrange("p (h c) -> p h c", c=NCH)
            m1 = smallp.tile([1, NH], F32, tag="m1")
            nc.vector.reduce_max(out=m1, in_=scv, axis=mybir.AxisListType.X)
            is1 = smallp.tile([1, NH * NCH], F32, tag="is1")
            nc.vector.tensor_tensor(
                is1[:, :].rearrange("p (h c) -> p h c", c=NCH),
                scv, m1[:, :, None].to_broadcast((1, NH, NCH)),
                mybir.AluOpType.is_ge)
            nc.vector.tensor_scalar_mul(is1, is1, 1e30)
            masked = smallp.tile([1, NH * NCH], F32, tag="masked")
            nc.vector.tensor_tensor(masked, sc, is1, mybir.AluOpType.subtract)
            m2 = smallp.tile([1, NH], F32, tag="m2")
            nc.vector.reduce_max(out=m2,
                                 in_=masked[:, :].rearrange("p (h c) -> p h c",
                                                            c=NCH),
                                 axis=mybir.AxisListType.X)
            is2 = smallp.tile([1, NH * NCH], F32, tag="is2")
            nc.vector.tensor_tensor(
                is2[:, :].rearrange("p (h c) -> p h c", c=NCH),
                masked[:, :].rearrange("p (h c) -> p h c", c=NCH),
                m2[:, :, None].to_broadcast((1, NH, NCH)),
                mybir.AluOpType.is_ge)
            iota4 = smallp.tile([1, NCH], F32, tag="iota4")
            nc.gpsimd.iota(iota4.bitcast(I32), pattern=[[1, NCH]],
                           base=0, channel_multiplier=0)
            nc.vector.tensor_copy(out=iota4, in_=iota4.bitcast(I32))
            nc.vector.tensor_scalar_mul(is1, is1, 1e-30)  # back to 0/1
            idxf = smallp.tile([1, NH, 2], F32, tag="idxf")
            w1 = smallp.tile([1, NH * NCH], F32, tag="w1")
            nc.vector.tensor_tensor(
                w1[:, :].rearrange("p (h c) -> p h c", c=NCH),
                is1[:, :].rearrange("p (h c) -> p h c", c=NCH),
                iota4[:, None, :].to_broadcast((1, NH, NCH)),
                mybir.AluOpType.mult)
            nc.vector.reduce_sum(out=idxf[:, :, 0], in_=w1[:, :].rearrange(
                "p (h c) -> p h c", c=NCH), axis=mybir.AxisListType.X)
            nc.vector.tensor_tensor(
                w1[:, :].rearrange("p (h c) -> p h c", c=NCH),
                is2[:, :].rearrange("p (h c) -> p h c", c=NCH),
                iota4[:, None, :].to_broadcast((1, NH, NCH)),
                mybir.AluOpType.mult)
            nc.vector.reduce_sum(out=idxf[:, :, 1], in_=w1[:, :].rearrange(
                "p (h c) -> p h c", c=NCH), axis=mybir.AxisListType.X)
            nc.vector.tensor_copy(out=idx_i32,
                                  in_=idxf[:, :, :].rearrange("p h s -> p (h s)"))

        if phases == "a":
            return _finish(nc)

        # ---------------- Phase B: attention ----------------
        otp = top.enter_context(tc.tile_pool(name="otp", bufs=1))
        oT_sb = otp.tile([128, NT, TOK], BF16)
        wop = top.enter_context(tc.tile_pool(name="wo", bufs=1))
        wo_sb = wop.tile([128, NT, D], BF16)
        for k in range(NT):
            nc.sync.dma_start(out=wo_sb[:, k, :], in_=woT.ap()[k])

        # dest-aligned DMA pieces for packing a 780-row source block into
        # the [128, NKT, 128] V tile at destination key offset k0
        def v_pieces(k0):
            out_p = []
            r = 0
            k = k0
            p0 = k % 128
            if p0:
                n = min(TOK, 128 - p0)
                out_p.append(("part", p0, k // 128, n, r))
                r += n
                k += n
            nk = (TOK - r) // 128
            if nk:
                out_p.append(("full", 0, k // 128, nk, r))
                r += nk * 128
                k += nk * 128
            if TOK - r:
                out_p.append(("part", 0, k // 128, TOK - r, r))
            return out_p

        with (
            tc.tile_pool(name="kv", bufs=2) as kvp,
            tc.tile_pool(name="ebf", bufs=4) as ep,
            tc.tile_pool(name="dacc", bufs=2) as dp,
            tc.tile_pool(name="bsm", bufs=2) as bsm,
            tc.tile_pool(name="pb_s", bufs=2, space="PSUM") as pbs,
            tc.tile_pool(name="pb_d", bufs=1, space="PSUM") as pbd,
            tc.tile_pool(name="pb_o", bufs=1, space="PSUM") as pbo,
        ):
            for h in range(NH):
                ta = 2 * (h // 2)
                r0 = (h % 2) * 64
                blk_regs = []
                for sel in range(2):
                    iv = nc.values_load(
                        idx_i32[0:1, h * 2 + sel: h * 2 + sel + 1],
                        min_val=0, max_val=NCH - 1,
                        skip_runtime_bounds_check=True)
                    blk_regs.append(iv)
                # repack this head's q channels (64 rows from each of the
                # pair tiles) into a single [128, TOK] tile via SBUF DMAs
                q_h = kvp.tile([128, TOK], BF16, tag="qh")
                nc.sync.dma_start(out=q_h[0:64, :],
                                  in_=qbf[r0:r0 + 64, ta, :])
                nc.sync.dma_start(out=q_h[64:128, :],
                                  in_=qbf[r0:r0 + 64, ta + 1, :])
                Kc = kvp.tile([128, NKT * 128], BF16, tag="kc")
                Vc = kvp.tile([128, NKT, 128], BF16, tag="vc")
                nc.vector.memset(Kc[:, SEL:], 0.0)
                nc.vector.memset(Vc[:, NKT - 1, :], 0.0)
                for sel in range(2):
                    for half in range(2):
                        blk = blk_regs[sel] * 2 + half
                        src = (ag_in.ap() if solo
                               else ag_out.ap()[_ds(blk, 1)][0])
                        kview = src[0].rearrange("(t p n) -> t p n",
                                                 p=128, n=TOK)
                        off = (sel * 2 + half) * TOK
                        nc.sync.dma_start(out=Kc[0:64, off:off + TOK],
                                          in_=kview[ta][r0:r0 + 64, :])
                        nc.sync.dma_start(out=Kc[64:128, off:off + TOK],
                                          in_=kview[ta + 1][r0:r0 + 64, :])
                        vview = src[1].rearrange("(t d) -> t d", d=D)
                        for kind, p0, kt0, n, vr0 in v_pieces(off):
                            vs = vview[vr0:vr0 + (n if kind == "part" else n * 128),
                                       h * HD:(h + 1) * HD]
                            if kind == "part":
                                nc.sync.dma_start(out=Vc[p0:p0 + n, kt0, :],
                                                  in_=vs)
                            else:
                                nc.sync.dma_start(
                                    out=Vc[:, kt0:kt0 + n, :],
                                    in_=vs.rearrange("(a p) d -> p a d", p=128))
                den = dp.tile([128, TOK], BF16, tag="den")
                nc.vector.memset(den, 0.0)
                pos = [pbo.tile([128, qn], F32, tag=f"po{qb}", name=f"po{qb}_{h}")
                       for qb, (q0, qn) in enumerate(QSA)]
                for kt in range(NKT):
                    ps = pbs.tile([128, TOK], F32, tag="ps")
                    for q0, qn in QSA:
                        nc.tensor.matmul(ps[:, q0:q0 + qn],
                                         lhsT=Kc[:, kt * 128:(kt + 1) * 128],
                                         rhs=q_h[:, q0:q0 + qn],
                                         start=True, stop=True)
                    ebf = ep.tile([128, TOK], BF16, tag="e")
                    bias_ap = pb_sb[:, 0:1] if kt == NKT - 1 else 0.0
                    nc.scalar.activation(out=ebf, in_=ps,
                                         func=mybir.ActivationFunctionType.Exp,
                                         bias=bias_ap, scale=SM_SCALE)
                    with nc.allow_low_precision(reason="softmax denom in bf16"):
                        nc.vector.tensor_tensor(den, den, ebf,
                                                mybir.AluOpType.add)
                    for qb, (q0, qn) in enumerate(QSA):
                        nc.tensor.matmul(pos[qb][:, :qn], lhsT=Vc[:, kt, :],
                                         rhs=ebf[:, q0:q0 + qn],
                                         start=(kt == 0), stop=(kt == NKT - 1))
                pdp = pbd.tile([1, TOK], F32, tag="pd")
                for q0, qn in QSA:
                    nc.tensor.matmul(pdp[:, q0:q0 + qn], lhsT=ones_col_bf,
                                     rhs=den[:, q0:q0 + qn],
                                     start=True, stop=True)
                rec = bsm.tile([1, TOK], F32R, tag="rec")
                with nc.allow_low_precision(reason="softmax denom"):
                    nc.vector.reciprocal(out=rec, in_=pdp)
                prb = pbs.tile([128, TOK], F32, tag="ps", name=f"prb{h}")
                for q0, qn in QSA:
                    nc.tensor.matmul(prb[:, q0:q0 + qn], lhsT=ones_row,
                                     rhs=rec[:, q0:q0 + qn],
                                     start=True, stop=True)
                rb_sb = bsm.tile([128, TOK], F32, tag="rb")
                nc.scalar.copy(out=rb_sb, in_=prb)
                for qb, (q0, qn) in enumerate(QSA):
                    with nc.allow_low_precision(reason="oT in bf16"):
                        nc.vector.tensor_tensor(oT_sb[:, h, q0:q0 + qn],
                                                pos[qb][:, :qn],
                                                rb_sb[:, q0:q0 + qn],
                                                mybir.AluOpType.mult)

        # ---------------- out projection ----------------
        if phases == "ab":
            return _finish(nc)
        with (
            tc.tile_pool(name="osb", bufs=3) as osb,
            tc.tile_pool(name="po_mm", bufs=2, space="PSUM") as po_mm,
        ):
            for tb in range(7):
                m = 128 if tb < 6 else MTAIL
                for nb in range(3):
                    pO = po_mm.tile([128, 512], F32, tag="pO")
                    for k in range(NT):
                        nc.tensor.matmul(pO[:m, :],
                                         lhsT=oT_sb[:, k, tb * 128: tb * 128 + m],
                                         rhs=wo_sb[:, k, nb * 512:(nb + 1) * 512],
                                         start=(k == 0), stop=(k == NT - 1))
                    ob = osb.tile([128, 512], F32, tag="ob")
                    nc.scalar.copy(out=ob[:m, :], in_=pO[:m, :])
                    nc.sync.dma_start(
                        out=out.ap()[tb * 128: tb * 128 + m,
                                     nb * 512:(nb + 1) * 512],
                        in_=ob[:m, :])

    return _finish(nc)


def _finish(nc):
    nc.compile()
    return nc


# ---------------- host-side prep ----------------

def _perm():
    # tile 2j rows: cos-halves (a) of heads 2j (rows 0:64) and 2j+1 (64:128);
    # tile 2j+1: the matching sin-halves (b)
    p = np.zeros(D, np.int64)
    for t in range(NT):
        j, s = t // 2, t % 2
        for r in range(128):
            h = 2 * j + r // 64
            c = r % 64
            p[t * 128 + r] = h * 128 + 2 * c + s
    return p


def make_fcis(freqs, grid_sizes):
    f, h, w = [int(v) for v in np.asarray(grid_sizes)[0]]
    c1 = C - 2 * (C // 3)
    c2 = C // 3
    fq = np.asarray(freqs, np.float32)
    ff = np.broadcast_to(fq[:f, None, None, :c1], (f, h, w, c1, 2))
    fh = np.broadcast_to(fq[None, :h, None, c1:c1 + c2], (f, h, w, c2, 2))
    fw = np.broadcast_to(fq[None, None, :w, c1 + c2:c1 + 2 * c2],
                         (f, h, w, c2, 2))
    return np.concatenate([ff, fh, fw], axis=3).reshape(f * h * w, C, 2)


def host_prep(inputs):
    """inputs: the full reference input dict -> per-core in_maps."""
    import ml_dtypes
    bf16 = ml_dtypes.bfloat16
    x = np.asarray(inputs["x"], np.float32)
    freqs = np.asarray(inputs["freqs"], np.float32)
    grid_sizes = np.asarray(inputs["grid_sizes"])
    assert x.shape == (1, S, D)
    assert int(np.asarray(inputs["chunk_size"])) == CHUNK
    assert int(np.asarray(inputs["top_k"])) == 2

    perm = _perm()
    wq = np.asarray(inputs["wq"], np.float32)[perm]
    wk = np.asarray(inputs["wk"], np.float32)[perm]
    wv = np.asarray(inputs["wv"], np.float32)
    wo = np.asarray(inputs["wo"], np.float32)
    gqv = np.asarray(inputs["gq"], np.float32)[perm]
    gkv = np.asarray(inputs["gk"], np.float32)[perm]
    for b in ("bq", "bk", "bv", "bo"):
        assert not np.any(np.asarray(inputs[b])), f"nonzero bias {b} unsupported"

    xT = np.ascontiguousarray(x[0].T.astype(bf16)).reshape(NT, 128, S)
    wqT = np.ascontiguousarray(wq.T.astype(bf16)).reshape(NT, 128, D)
    wkT = np.ascontiguousarray(wk.T.astype(bf16)).reshape(NT, 128, D)
    wvT = np.ascontiguousarray(wv.T.astype(bf16)).reshape(NT, 128, D)
    woT = np.ascontiguousarray(wo.T.astype(bf16)).reshape(NT, 128, D)

    fcis = make_fcis(freqs, grid_sizes)  # [S, C, 2]
    frT = fcis[:, :, 0].T  # [C, S]
    fiT = fcis[:, :, 1].T

    pad_bias = np.zeros((128, 1), np.float32)
    pad_bias[LASTK:] = -1e30
    in_maps = []
    for c in range(N_CORES):
        sl = slice(c * TOK, (c + 1) * TOK)
        cm = np.zeros((128, NCH), np.float32)
        cm[:, c // 2] = 1.0
        in_maps.append({
            "xT": np.ascontiguousarray(xT[:, :, sl]),
            "wqT": wqT, "wkT": wkT, "wvT": wvT, "woT": woT,
            "gq": np.ascontiguousarray(gqv.reshape(NT, 128).T),
            "gk": np.ascontiguousarray(gkv.reshape(NT, 128).T),
            "f1": np.ascontiguousarray(np.concatenate(
                [frT[:, sl], frT[:, sl]], axis=0)),
            "f2": np.ascontiguousarray(np.concatenate(
                [fiT[:, sl], fiT[:, sl]], axis=0)),
            "chmask": cm,
            "padbias": pad_bias,
        })
    return in_maps


def assemble_out(results):
    return np.concatenate([r["out"] for r in results], axis=0)[None]


# ---------------- harness entry point ----------------

_CACHE = {}


def kernel(**inputs):
    import numpy as _np
    ug = bool(_np.all(_np.asarray(inputs["gq"]) == 1.0)
              and _np.all(_np.asarray(inputs["gk"]) == 1.0))
    key = ("nc", ug)
    if key not in _CACHE:
        _CACHE[key] = build_kernel(unit_gains=ug)
    nc = _CACHE[key]
    in_maps = host_prep(inputs)
    from concourse import bass_utils
    res = bass_utils.run_bass_kernel_spmd(
        nc, in_maps, core_ids=list(range(N_CORES)), trace=False)
    return assemble_out(res.results).astype(_np.float32)
